# revision 2
# baseline (speedup 1.0000x reference)
"""Trainium2 Bass kernel for nn_LoadPathLoss (v3: raw bass, bf16, minimal sync).

reference computation:
  structure = state[:, ch]                  # [B=4, D=64, H=128, W=128]
  s = structure[:, 0]
  for z in 1..63:  s = min(maxpool3x3(s), max(s, structure[:, z]))
  return relu(structure - s[:, None]).mean()

Strategy: pure data parallel over B=4, one batch element per core. Per step,
the 3x3 pool is computed W-direction-first on DVE over a padded bf16 S tile,
then the H-direction 3-max comes from two PE shift-matmuls (bf16; the shift
matrices carry self-copies in their boundary columns, so no -inf offset is
needed and the scan is exact over bf16-quantized inputs). Engine queues are
kept in-order with cross-engine semaphores only. The Act engine converts
chunks fp32->bf16 for phase 2 off the critical path; phase 2 reduces
sum(max(c, s_final)) with DVE maxes + PE ones-matmul column sums in PSUM.
"""

import numpy as np

B, C, D, H, W = 4, 8, 64, 128, 128
NCORES = 4
ZCHUNK = 8
NCHUNK = D // ZCHUNK
NEG = -1000.0

_cached = {}


def _build_nc(d_steps=D):
    import concourse.bacc as bacc
    import concourse.mybir as mybir

    nc = bacc.Bacc("TRN2", target_bir_lowering=False, debug=False)
    fp32 = mybir.dt.float32
    bf16 = mybir.dt.bfloat16
    A = mybir.AluOpType
    mx, mn = A.max, A.min

    cb = nc.dram_tensor("cb", [D, H, W], fp32, kind="ExternalInput")
    # cols 0:H = U' (up-shift, self at edge), H:2H = D' (down-shift), 2H = ones
    wts = nc.dram_tensor("wts", [H, 2 * H + 2], bf16, kind="ExternalInput")
    out = nc.dram_tensor("out", [1, 2], fp32, kind="ExternalOutput")

    cf32 = [nc.alloc_sbuf_tensor(f"cf32_{k}", [H, ZCHUNK, W], fp32) for k in range(NCHUNK)]
    c16 = [nc.alloc_sbuf_tensor(f"c16_{k}", [H, ZCHUNK, W], bf16) for k in range(NCHUNK)]
    wtile = nc.alloc_sbuf_tensor("wtile", [H, 2 * H + 2], bf16)
    S = nc.alloc_sbuf_tensor("S", [H, W + 2], bf16)       # padded, data in 1:W+1
    m = nc.alloc_sbuf_tensor("m", [H, W], bf16)
    t129 = nc.alloc_sbuf_tensor("t129", [H, W + 1], bf16)
    w3s = nc.alloc_sbuf_tensor("w3s", [H, W], bf16)       # maxW3(S), rhs of mms
    hp1 = nc.alloc_sbuf_tensor("hp1", [H, W], bf16)
    zdum = nc.alloc_sbuf_tensor("zdum", [H, 8], bf16)
    red = nc.alloc_sbuf_tensor("red", [1, 2], fp32)
    ps = nc.alloc_psum_tensor("ps", [H, 2 * W], fp32)
    ps2 = nc.alloc_psum_tensor("ps2", [1, 512], fp32)
    ps3 = nc.alloc_psum_tensor("ps3", [1, W], fp32)

    dma_sem = nc.alloc_semaphore("dma_sem")
    conv_sem = nc.alloc_semaphore("conv_sem")
    pe_sem = nc.alloc_semaphore("pe_sem")
    w3_sem = nc.alloc_semaphore("w3_sem")
    dve_sem = nc.alloc_semaphore("dve_sem")
    p2_sem = nc.alloc_semaphore("p2_sem")
    pe2_sem = nc.alloc_semaphore("pe2_sem")
    dum_sem = nc.alloc_semaphore("dum_sem")

    with nc.Block() as blk:
        @blk.sync
        def _(sync):
            # chunk 0 split for fast scan start: z0..1 then z2..7
            sync.dma_start(
                cf32[0][:, 0:2, :], cb[0:2].rearrange("z h w -> h z w")
            ).then_inc(dma_sem, 16)
            sync.dma_start(
                cf32[0][:, 2:ZCHUNK, :], cb[2:ZCHUNK].rearrange("z h w -> h z w")
            ).then_inc(dma_sem, 16)
            sync.dma_start(wtile[:], wts[:, :]).then_inc(dma_sem, 16)
            for k in range(1, NCHUNK):
                src = cb[k * ZCHUNK : (k + 1) * ZCHUNK].rearrange("z h w -> h z w")
                sync.dma_start(cf32[k][:], src).then_inc(dma_sem, 16)
            # final output DMA
            sync.wait_ge(dve_sem, 2)
            sync.dma_start(out[:, :], red[:]).then_inc(dma_sem, 16)

        @blk.scalar
        def _(scalar):
            # conv chunks fp32 -> bf16 (only phase 2 consumes these)
            scalar.wait_ge(dma_sem, 32)
            nc.scalar.copy(c16[0][:], cf32[0][:]).then_inc(conv_sem, 1)
            for k in range(1, NCHUNK):
                scalar.wait_ge(dma_sem, 16 * (k + 3))
                nc.scalar.copy(c16[k][:], cf32[k][:]).then_inc(conv_sem, 1)

        @blk.tensor
        def _(tensor):
            # warmup: pin pe_busy_start early so the scan runs at full clock
            tensor.wait_ge(dum_sem, 1)
            nc.tensor.matmul(out=ps[0:8, 0:8], lhsT=zdum[:], rhs=zdum[:], start=True, stop=True)
            nc.tensor.matmul(out=ps[0:8, 0:8], lhsT=zdum[:], rhs=zdum[:], start=True, stop=True)
            tensor.wait_ge(dma_sem, 48)  # weights loaded
            for z in range(1, d_steps):
                tensor.wait_ge(w3_sem, z)
                nc.tensor.matmul(
                    out=ps[:, 0:W], lhsT=wtile[:, 0:H], rhs=w3s[:],
                    start=True, stop=True,
                ).then_inc(pe_sem, 1)
                nc.tensor.matmul(
                    out=ps[:, W : 2 * W], lhsT=wtile[:, H : 2 * H], rhs=w3s[:],
                    start=True, stop=True,
                ).then_inc(pe_sem, 1)
            # sum of s over h (first phase-2 max implies the last min is done)
            tensor.wait_ge(p2_sem, 1)
            nc.tensor.matmul(
                out=ps3[:, :], lhsT=wtile[:, 2 * H : 2 * H + 1],
                rhs=S[:, 1 : W + 1], start=True, stop=True,
            ).then_inc(pe2_sem, 1)
            # phase 2: column sums of maxed chunks via ones-matmuls into ps2
            for k in range(NCHUNK):
                tensor.wait_ge(p2_sem, k + 1)
                nc.tensor.matmul(
                    out=ps2[:, :], lhsT=wtile[:, 2 * H : 2 * H + 1],
                    rhs=c16[k][:, 0 : ZCHUNK // 2, :],
                    start=(k == 0), stop=False,
                )
                nc.tensor.matmul(
                    out=ps2[:, :], lhsT=wtile[:, 2 * H : 2 * H + 1],
                    rhs=c16[k][:, ZCHUNK // 2 : ZCHUNK, :],
                    start=False, stop=(k == NCHUNK - 1),
                ).then_inc(pe2_sem, 1)

        @blk.vector
        def _(vector):
            nc.vector.memset(zdum[:], 0.0).then_inc(dum_sem, 1)
            nc.vector.memset(S[:, 0:1], NEG)
            nc.vector.memset(S[:, W + 1 : W + 2], NEG)
            vector.wait_ge(dma_sem, 16)
            nc.vector.tensor_copy(S[:, 1 : W + 1], cf32[0][:, 0, :])
            for z in range(1, d_steps):
                k, j = z // ZCHUNK, z % ZCHUNK
                # W-direction pool of S (padded -> exact)
                nc.vector.tensor_tensor(
                    out=t129[:], in0=S[:, 0 : W + 1], in1=S[:, 1 : W + 2], op=mx
                )
                nc.vector.tensor_tensor(
                    out=w3s[:], in0=t129[:, 0:W], in1=S[:, 2 : W + 2], op=mx
                ).then_inc(w3_sem, 1)
                # m = max(S, c_z) straight off the fp32 chunk (hidden under
                # the PE round-trip window; Act conversions are off-path)
                if z == 2:
                    vector.wait_ge(dma_sem, 32)
                elif j == 0 and k > 0:
                    vector.wait_ge(dma_sem, 16 * (k + 3))
                nc.vector.tensor_tensor(
                    out=m[:], in0=S[:, 1 : W + 1], in1=cf32[k][:, j, :], op=mx
                )
                # H-direction pool from PSUM shifts, then the step min
                vector.wait_ge(pe_sem, 2 * z - 1)
                nc.vector.tensor_tensor(
                    out=hp1[:], in0=w3s[:], in1=ps[:, 0:W], op=mx
                )
                vector.wait_ge(pe_sem, 2 * z)
                nc.vector.tensor_tensor(
                    out=hp1[:], in0=hp1[:], in1=ps[:, W : 2 * W], op=mx
                )
                nc.vector.tensor_tensor(
                    out=S[:, 1 : W + 1], in0=hp1[:], in1=m[:], op=mn
                )
            # phase 2: maxed chunks (in place), PE consumes them
            sbc = S[:, 1 : W + 1].unsqueeze(1).broadcast_to((H, ZCHUNK, W))
            for k in range(NCHUNK):
                vector.wait_ge(conv_sem, k + 1)
                nc.vector.tensor_tensor(
                    out=c16[k][:], in0=c16[k][:], in1=sbc, op=mx
                ).then_inc(p2_sem, 1)
            # final reductions of the PE column sums
            vector.wait_ge(pe2_sem, NCHUNK + 1)
            nc.vector.tensor_reduce(
                out=red[:, 0:1], in_=ps2[:, :], axis=mybir.AxisListType.X, op=A.add
            ).then_inc(dve_sem, 1)
            nc.vector.tensor_reduce(
                out=red[:, 1:2], in_=ps3[:, :], axis=mybir.AxisListType.X, op=A.add
            ).then_inc(dve_sem, 1)

    nc.compile()
    return nc


def _shift_weights():
    import ml_dtypes
    Wm = np.zeros((H, 2 * H + 2), dtype=np.float32)
    for p in range(H - 1):
        Wm[p + 1, p] = 1.0            # U': ps_up[p] = x[p+1]
        Wm[p, H + p + 1] = 1.0        # D': ps_dn[p] = x[p-1]
    Wm[H - 1, H - 1] = 1.0            # U' edge: self
    Wm[0, H + 0] = 1.0                # D' edge: self
    Wm[:, 2 * H] = 1.0                # ones column
    return Wm.astype(ml_dtypes.bfloat16)


def _make_runner(nc):
    """Cached multi-core PJRT runner (mirrors bass2jax.run_bass_via_pjrt but
    keeps the jitted shard_map so repeat calls skip retrace/recompile)."""
    import jax
    from jax.sharding import Mesh, PartitionSpec
    from jax.experimental.shard_map import shard_map
    import concourse.mybir as mybir
    from concourse import bass2jax

    bass2jax.install_neuronx_cc_hook()

    partition_name = nc.partition_id_tensor.name if nc.partition_id_tensor else None
    in_names, out_names, out_avals, zero_outs = [], [], [], []
    for alloc in nc.m.functions[0].allocations:
        if not isinstance(alloc, mybir.MemoryLocationSet):
            continue
        name = alloc.memorylocations[0].name
        if alloc.kind == "ExternalInput":
            if name != partition_name:
                in_names.append(name)
        elif alloc.kind == "ExternalOutput":
            shape = tuple(alloc.tensor_shape)
            dtype = mybir.dt.np(alloc.dtype)
            out_names.append(name)
            out_avals.append(jax.core.ShapedArray(shape, dtype))
            zero_outs.append(np.zeros(shape, dtype))
    n_params = len(in_names)
    n_outs = len(out_avals)
    all_names = in_names + out_names
    donate = tuple(range(n_params, n_params + n_outs))

    def _body(*args):
        operands = list(args)
        if partition_name is not None:
            operands.append(bass2jax.partition_id_tensor())
        outs = bass2jax._bass_exec_p.bind(
            *operands,
            out_avals=tuple(out_avals),
            in_names=tuple(all_names + ([partition_name] if partition_name else [])),
            out_names=tuple(out_names),
            lowering_input_output_aliases=(),
            sim_require_finite=True,
            sim_require_nnan=True,
            nc=nc,
        )
        return tuple(outs)

    devices = jax.devices()[:NCORES]
    mesh = Mesh(np.asarray(devices), ("core",))
    in_specs = (PartitionSpec("core"),) * (n_params + n_outs)
    out_specs = (PartitionSpec("core"),) * n_outs
    sharded = jax.jit(
        shard_map(_body, mesh=mesh, in_specs=in_specs, out_specs=out_specs,
                  check_rep=False),
        donate_argnums=donate, keep_unused=True,
    )

    def run(in_maps):
        args = [
            np.concatenate([np.asarray(m[name]) for m in in_maps], axis=0)
            for name in in_names
        ]
        zouts = [np.concatenate([z] * NCORES, axis=0) for z in zero_outs]
        outs = sharded(*args, *zouts)
        res = []
        for b in range(NCORES):
            d = {}
            for i, name in enumerate(out_names):
                full = np.asarray(outs[i])
                per = full.shape[0] // NCORES
                d[name] = full[b * per : (b + 1) * per]
            res.append(d)
        return res

    return run


def kernel(state, ch_structure):
    if "nc" not in _cached:
        _cached["nc"] = _build_nc()
        _cached["run"] = _make_runner(_cached["nc"])

    structure = np.ascontiguousarray(state[:, int(ch_structure)], dtype=np.float32)
    wm = _shift_weights()
    in_maps = [{"cb": structure[b], "wts": wm} for b in range(NCORES)]
    results = _cached["run"](in_maps)
    _cached["last"] = results

    total = 0.0
    for b in range(NCORES):
        o = np.asarray(results[b]["out"], dtype=np.float64)
        total += o[0, 0] - float(D) * o[0, 1]
    mean = total / float(B * D * H * W)
    return np.asarray(mean, dtype=np.float32)


if __name__ == "__main__":
    rng = np.random.default_rng(0)
    st = rng.standard_normal((B, C, D, H, W)).astype(np.float32)
    print(kernel(st, 3))


# revision 4
# speedup vs baseline: 1124.0616x; 1124.0616x over previous
"""Trainium2 Bass kernel for nn_LoadPathLoss (v4).

reference computation:
  structure = state[:, ch]                  # [B=4, D=64, H=128, W=128]
  s = structure[:, 0]
  for z in 1..63:  s = min(maxpool3x3(s), max(s, structure[:, z]))
  return relu(structure - s[:, None]).mean()

Strategy: 8 cores = 4 batch elements x 2 mirrored W-halves. Each core scans
its half with a shrinking halo (width 64 + remaining-steps), so no mid-scan
communication is needed; odd cores get W-mirrored input (the pool commutes
with mirroring) and every core runs the identical SPMD program. Per step the
3x3 pool is W-direction-first on DVE over a padded bf16 S tile; the
H-direction 3-max comes from two PE shift-matmuls (bf16, boundary columns
are self-copies so no -inf offset is needed; the scan is exact over
bf16-quantized inputs), folded by two single-PSUM-operand maxes. Engine
queues stay in-order with cross-engine semaphores only. Act converts chunks
fp32->bf16 off the critical path; phase 2 reduces sum(max(c, s_final)) over
each core's valid half via DVE maxes + PE ones-matmul column sums in PSUM.
"""

import numpy as np

B, C, D, H, W = 4, 8, 64, 128, 128
HW = W // 2
ZCHUNK = 8
NCHUNK = D // ZCHUNK
NEG = -1000.0


def nwidth(z):
    # valid S width (cols 0..n-1) needed after step z
    return min(W - 1, HW + (D - 1 - z))


def _build_nc(d_steps=D):
    import concourse.bacc as bacc
    import concourse.mybir as mybir

    nc = bacc.Bacc("TRN2", target_bir_lowering=False, debug=False)
    fp32 = mybir.dt.float32
    bf16 = mybir.dt.bfloat16
    A = mybir.AluOpType
    mx, mn = A.max, A.min

    cb = nc.dram_tensor("cb", [D, H, W], fp32, kind="ExternalInput")
    wts = nc.dram_tensor("wts", [H, 2 * H + 2], bf16, kind="ExternalInput")
    out = nc.dram_tensor("out", [1, 2], fp32, kind="ExternalOutput")

    cf32 = [nc.alloc_sbuf_tensor(f"cf32_{k}", [H, ZCHUNK, W], fp32) for k in range(NCHUNK)]
    c16 = [nc.alloc_sbuf_tensor(f"c16_{k}", [H, ZCHUNK, HW], bf16) for k in range(NCHUNK)]
    wtile = nc.alloc_sbuf_tensor("wtile", [H, 2 * H + 2], bf16)
    S = nc.alloc_sbuf_tensor("S", [H, W + 2], bf16)
    m = nc.alloc_sbuf_tensor("m", [H, W], bf16)
    t129 = nc.alloc_sbuf_tensor("t129", [H, W + 1], bf16)
    w3s = nc.alloc_sbuf_tensor("w3s", [H, W], bf16)
    hp1 = nc.alloc_sbuf_tensor("hp1", [H, W], bf16)
    zdum = nc.alloc_sbuf_tensor("zdum", [H, 8], bf16)
    red = nc.alloc_sbuf_tensor("red", [1, 2], fp32)
    ps = nc.alloc_psum_tensor("ps", [H, 2 * W], fp32)
    ps2 = nc.alloc_psum_tensor("ps2", [1, ZCHUNK * HW], fp32)
    ps3 = nc.alloc_psum_tensor("ps3", [1, HW], fp32)

    dma_sem = nc.alloc_semaphore("dma_sem")
    conv_sem = nc.alloc_semaphore("conv_sem")
    pe_sem = nc.alloc_semaphore("pe_sem")
    w3_sem = nc.alloc_semaphore("w3_sem")
    dve_sem = nc.alloc_semaphore("dve_sem")
    p2_sem = nc.alloc_semaphore("p2_sem")
    pe2_sem = nc.alloc_semaphore("pe2_sem")
    dum_sem = nc.alloc_semaphore("dum_sem")

    with nc.Block() as blk:
        @blk.sync
        def _(sync):
            sync.dma_start(
                cf32[0][:, 0:2, :], cb[0:2].rearrange("z h w -> h z w")
            ).then_inc(dma_sem, 16)
            sync.dma_start(
                cf32[0][:, 2:ZCHUNK, :], cb[2:ZCHUNK].rearrange("z h w -> h z w")
            ).then_inc(dma_sem, 16)
            sync.dma_start(wtile[:], wts[:, :]).then_inc(dma_sem, 16)
            for k in range(1, NCHUNK):
                src = cb[k * ZCHUNK : (k + 1) * ZCHUNK].rearrange("z h w -> h z w")
                sync.dma_start(cf32[k][:], src).then_inc(dma_sem, 16)
            sync.wait_ge(dve_sem, 2)
            sync.dma_start(out[:, :], red[:]).then_inc(dma_sem, 16)

        @blk.scalar
        def _(scalar):
            # convert only the valid half for phase 2
            scalar.wait_ge(dma_sem, 32)
            nc.scalar.copy(c16[0][:], cf32[0][:, :, 0:HW]).then_inc(conv_sem, 1)
            for k in range(1, NCHUNK):
                scalar.wait_ge(dma_sem, 16 * (k + 3))
                nc.scalar.copy(c16[k][:], cf32[k][:, :, 0:HW]).then_inc(conv_sem, 1)

        @blk.tensor
        def _(tensor):
            tensor.wait_ge(dum_sem, 1)
            nc.tensor.matmul(out=ps[0:8, 0:8], lhsT=zdum[:], rhs=zdum[:], start=True, stop=True)
            nc.tensor.matmul(out=ps[0:8, 0:8], lhsT=zdum[:], rhs=zdum[:], start=True, stop=True)
            tensor.wait_ge(dma_sem, 48)
            for z in range(1, d_steps):
                n = nwidth(z)
                tensor.wait_ge(w3_sem, z)
                nc.tensor.matmul(
                    out=ps[:, 0:n], lhsT=wtile[:, 0:H], rhs=w3s[:, 0:n],
                    start=True, stop=True,
                ).then_inc(pe_sem, 1)
                nc.tensor.matmul(
                    out=ps[:, W : W + n], lhsT=wtile[:, H : 2 * H], rhs=w3s[:, 0:n],
                    start=True, stop=True,
                ).then_inc(pe_sem, 1)
            tensor.wait_ge(p2_sem, 1)
            nc.tensor.matmul(
                out=ps3[:, :], lhsT=wtile[:, 2 * H : 2 * H + 1],
                rhs=S[:, 1 : HW + 1], start=True, stop=True,
            ).then_inc(pe2_sem, 1)
            for k in range(NCHUNK):
                tensor.wait_ge(p2_sem, k + 1)
                nc.tensor.matmul(
                    out=ps2[:, :], lhsT=wtile[:, 2 * H : 2 * H + 1],
                    rhs=c16[k][:],
                    start=(k == 0), stop=(k == NCHUNK - 1),
                ).then_inc(pe2_sem, 1)

        @blk.vector
        def _(vector):
            nc.vector.memset(zdum[:], 0.0).then_inc(dum_sem, 1)
            nc.vector.memset(S[:, 0:1], NEG)
            vector.wait_ge(dma_sem, 16)
            nc.vector.tensor_copy(S[:, 1:W], cf32[0][:, 0, 0 : W - 1])
            for z in range(1, d_steps):
                n = nwidth(z)
                k, j = z // ZCHUNK, z % ZCHUNK
                nc.vector.tensor_tensor(
                    out=t129[:, 0 : n + 1], in0=S[:, 0 : n + 1], in1=S[:, 1 : n + 2], op=mx
                )
                nc.vector.tensor_tensor(
                    out=w3s[:, 0:n], in0=t129[:, 0:n], in1=S[:, 2 : n + 2], op=mx
                ).then_inc(w3_sem, 1)
                if z == 2:
                    vector.wait_ge(dma_sem, 32)
                elif j == 0 and k > 0:
                    vector.wait_ge(dma_sem, 16 * (k + 3))
                nc.vector.tensor_tensor(
                    out=m[:, 0:n], in0=S[:, 1 : n + 1], in1=cf32[k][:, j, 0:n], op=mx
                )
                vector.wait_ge(pe_sem, 2 * z - 1)
                nc.vector.tensor_tensor(
                    out=hp1[:, 0:n], in0=w3s[:, 0:n], in1=ps[:, 0:n], op=mx
                )
                vector.wait_ge(pe_sem, 2 * z)
                nc.vector.tensor_tensor(
                    out=hp1[:, 0:n], in0=hp1[:, 0:n], in1=ps[:, W : W + n], op=mx
                )
                nc.vector.tensor_tensor(
                    out=S[:, 1 : n + 1], in0=hp1[:, 0:n], in1=m[:, 0:n], op=mn
                )
            sbc = S[:, 1 : HW + 1].unsqueeze(1).broadcast_to((H, ZCHUNK, HW))
            for k in range(NCHUNK):
                vector.wait_ge(conv_sem, k + 1)
                nc.vector.tensor_tensor(
                    out=c16[k][:], in0=c16[k][:], in1=sbc, op=mx
                ).then_inc(p2_sem, 1)
            vector.wait_ge(pe2_sem, NCHUNK + 1)
            nc.vector.tensor_reduce(
                out=red[:, 0:1], in_=ps2[:, :], axis=mybir.AxisListType.X, op=A.add
            ).then_inc(dve_sem, 1)
            nc.vector.tensor_reduce(
                out=red[:, 1:2], in_=ps3[:, :], axis=mybir.AxisListType.X, op=A.add
            ).then_inc(dve_sem, 1)

    nc.compile()
    return nc


def _shift_weights():
    import ml_dtypes
    Wm = np.zeros((H, 2 * H + 2), dtype=np.float32)
    for p in range(H - 1):
        Wm[p + 1, p] = 1.0
        Wm[p, H + p + 1] = 1.0
    Wm[H - 1, H - 1] = 1.0
    Wm[0, H + 0] = 1.0
    Wm[:, 2 * H] = 1.0
    return Wm.astype(ml_dtypes.bfloat16)



NCORES = 8
_cached = {}

def _make_runner(nc):
    """Cached multi-core PJRT runner (mirrors bass2jax.run_bass_via_pjrt but
    keeps the jitted shard_map so repeat calls skip retrace/recompile)."""
    import jax
    from jax.sharding import Mesh, PartitionSpec
    from jax.experimental.shard_map import shard_map
    import concourse.mybir as mybir
    from concourse import bass2jax

    bass2jax.install_neuronx_cc_hook()

    partition_name = nc.partition_id_tensor.name if nc.partition_id_tensor else None
    in_names, out_names, out_avals, zero_outs = [], [], [], []
    for alloc in nc.m.functions[0].allocations:
        if not isinstance(alloc, mybir.MemoryLocationSet):
            continue
        name = alloc.memorylocations[0].name
        if alloc.kind == "ExternalInput":
            if name != partition_name:
                in_names.append(name)
        elif alloc.kind == "ExternalOutput":
            shape = tuple(alloc.tensor_shape)
            dtype = mybir.dt.np(alloc.dtype)
            out_names.append(name)
            out_avals.append(jax.core.ShapedArray(shape, dtype))
            zero_outs.append(np.zeros(shape, dtype))
    n_params = len(in_names)
    n_outs = len(out_avals)
    all_names = in_names + out_names
    donate = tuple(range(n_params, n_params + n_outs))

    def _body(*args):
        operands = list(args)
        if partition_name is not None:
            operands.append(bass2jax.partition_id_tensor())
        outs = bass2jax._bass_exec_p.bind(
            *operands,
            out_avals=tuple(out_avals),
            in_names=tuple(all_names + ([partition_name] if partition_name else [])),
            out_names=tuple(out_names),
            lowering_input_output_aliases=(),
            sim_require_finite=True,
            sim_require_nnan=True,
            nc=nc,
        )
        return tuple(outs)

    devices = jax.devices()[:NCORES]
    mesh = Mesh(np.asarray(devices), ("core",))
    in_specs = (PartitionSpec("core"),) * (n_params + n_outs)
    out_specs = (PartitionSpec("core"),) * n_outs
    sharded = jax.jit(
        shard_map(_body, mesh=mesh, in_specs=in_specs, out_specs=out_specs,
                  check_rep=False),
        donate_argnums=donate, keep_unused=True,
    )

    def run(in_maps):
        args = [
            np.concatenate([np.asarray(m[name]) for m in in_maps], axis=0)
            for name in in_names
        ]
        zouts = [np.concatenate([z] * NCORES, axis=0) for z in zero_outs]
        outs = sharded(*args, *zouts)
        res = []
        for b in range(NCORES):
            d = {}
            for i, name in enumerate(out_names):
                full = np.asarray(outs[i])
                per = full.shape[0] // NCORES
                d[name] = full[b * per : (b + 1) * per]
            res.append(d)
        return res

    return run


def kernel(state, ch_structure):
    if "nc" not in _cached:
        _cached["nc"] = _build_nc()
        _cached["run"] = _make_runner(_cached["nc"])

    structure = np.ascontiguousarray(state[:, int(ch_structure)], dtype=np.float32)
    wm = _shift_weights()
    in_maps = []
    for b in range(B):
        in_maps.append({"cb": structure[b], "wts": wm})
        in_maps.append(
            {"cb": np.ascontiguousarray(structure[b][:, :, ::-1]), "wts": wm}
        )
    results = _cached["run"](in_maps)

    total = 0.0
    for r in results:
        o = np.asarray(r["out"], dtype=np.float64)
        total += o[0, 0] - float(D) * o[0, 1]
    mean = total / float(B * D * H * W)
    return np.asarray(mean, dtype=np.float32)


if __name__ == "__main__":
    rng = np.random.default_rng(0)
    st = rng.standard_normal((B, C, D, H, W)).astype(np.float32)
    print(kernel(st, 3))
